# revision 29
# baseline (speedup 1.0000x reference)
"""GAT layer kernel for Trainium2, 8-core data-parallel over batch.

Math (per batch b, head h):
    h = x @ W                              [N, H*HD]
    s_n = <h[n, hHD:(h+1)HD], a_src[h]>,  t_n likewise with a_dst
    A[j, i] = exp(leakyrelu(s_i + t_j, 0.2)),  out[i] = (sum_j A[j,i] h_j) / Z_i

Key identities (softmax over j is invariant to per-i factors; per-j factors
can be folded into the aggregation weights):
    A''[j, i] = max(exp(0.8 s_i), exp(-0.8 t_j))        -- ONE max op per tile
    wt'[j, c] = [h_j | 1][c] * exp(t_j - K)             -- folded j factor
    sum_j wt'[j,c] A''[j,i] = e^{-K-0.2 s_i} sum_j A[j,i] [h_j | 1][c]
so the [33, N] PSUM accumulators hold num and Z up to a per-column factor
that cancels in num/Z. K re-centers ln Z into the ACT Ln LUT's accurate
range (it breaks down above ~40).

Engine plan per core (= one batch element):
  - PE: st rows + hT with stationary weights (f32r), 8 tiny transposes for
    the exp(t) columns, 64 bulk matmuls, head-3's K=1 1/Z broadcast.
  - DVE: 28 of 32 attention tiles (tensor_scalar_max, bf16 4x mode),
    wt scaling, per-head normalize multiplies.
  - ACT: all exps (one scale/bias-AP'd op covers exp(t-K) and exp(-0.8t)),
    hT PSUM->SBUF copy, 4 attention tiles via double-Relu, per-head
    1/Z = Exp(-Ln(Z)) read straight from PSUM.
  - GpSimd: DMA queue driving only (its tensor ops are microcoded-slow AND
    starve DVE through the shared SBUF port).
  - wt transposes ride the DMA xbar transpose engine (SBUF->SBUF).
  - A single ACT table set (natural_log_exp_and_others) serves Copy/Exp/Ln/
    Relu so the tail doesn't thrash table loads (1.28us each).
  - per-head normalize tails are pipelined one head behind the bulk loop;
    head 3 avoids DMA latency via a K=1 PE matmul broadcast of 1/Z.
"""

import numpy as np

B, N, IN_F, OUT_F, H = 8, 1024, 128, 128, 4
HD = OUT_F // H  # 32
NEG = 0.2
K_SHIFT = 14.0  # ln-space downshift of the accumulators (cancels in num/Z)
N_CORES = 8
NT = N // 128  # 8 node tiles

ACT_TILES = {(0, 6), (1, 4)}  # built on ACT (double-Relu) in its idle window

_CACHE = {}


def _patch_act_tables():
    """Restrict Copy/Exp/Ln/Relu/ParametricRelu to the one table set that
    holds them all, so the scheduler emits a single ACT_TABLE_LOAD instead
    of thrashing between per-function tables (1.28us per reload)."""
    import concourse.bacc as bacc_mod
    if getattr(bacc_mod, "_gat_tables_patched", False):
        return
    orig = bacc_mod.get_activation_tables

    def patched(arch):
        tables = orig(arch)
        combined = "natural_log_exp_and_others"
        if combined in tables:
            keep = tables[combined]
            for name, funcs in tables.items():
                if name != combined:
                    funcs -= keep
        return tables

    bacc_mod.get_activation_tables = patched
    bacc_mod._gat_tables_patched = True


def _build_nc():
    _patch_act_tables()
    import concourse.bacc as bacc
    import concourse.tile as tile
    from concourse import mybir
    from concourse.masks import make_identity

    f32 = mybir.dt.float32
    f32r = mybir.dt.float32r
    bf16 = mybir.dt.bfloat16
    AF = mybir.ActivationFunctionType
    ALU = mybir.AluOpType

    nc = bacc.Bacc("TRN2", target_bir_lowering=False, debug=False,
                   num_devices=N_CORES)

    xT = nc.declare_dram_parameter("xT", [IN_F, N], f32, isOutput=False)
    Wd = nc.declare_dram_parameter("W", [IN_F, OUT_F], f32, isOutput=False)
    # Wa_ext = W @ [a_dst | a_dst | 0... | a_src]: st rows 0-7 = t twice,
    # rows 32-35 = s (32-aligned partition offsets for the ACT reads)
    Wa = nc.declare_dram_parameter("Wa", [IN_F, 36], f32, isOutput=False)
    scv_d = nc.declare_dram_parameter("scv", [36, 2], f32, isOutput=False)
    outT = nc.declare_dram_parameter("outT", [OUT_F, N], f32, isOutput=True)

    e8s_dram = nc.dram_tensor("e8s_scratch", [H, N], bf16)
    rz16_dram = nc.dram_tensor("rz16_scratch", [2, N], bf16)

    with tile.TileContext(nc) as tc:
      with (
        tc.tile_pool(name="const", bufs=1) as cpool,
        tc.tile_pool(name="atile", bufs=10) as apool,
        tc.tile_pool(name="tail", bufs=2) as tpool,
      ):
        with tc.tile_pool(name="ps_pre", bufs=1, space="PSUM") as pspre:
            # ---- load inputs ----
            xT_sb = cpool.tile([IN_F, N], f32, tag="xT")
            nc.sync.dma_start(out=xT_sb[:, 0:512], in_=xT[:, 0:512])
            nc.gpsimd.dma_start(out=xT_sb[:, 512:N], in_=xT[:, 512:N])
            W_sb = cpool.tile([IN_F, OUT_F], f32, tag="W")
            nc.sync.dma_start(out=W_sb, in_=Wd[:])
            Wa_sb = cpool.tile([IN_F, 36], f32, tag="Wa")
            nc.sync.dma_start(out=Wa_sb, in_=Wa[:])
            scvec = cpool.tile([36, 2], f32, tag="scvec")
            nc.sync.dma_start(out=scvec, in_=scv_d[:])

            # f32->f32r rounding casts (fp32 matmul is 1/4 rate; f32r
            # streams 1 col/cyc and the verifier wants explicit rounding)
            xTr = cpool.tile([IN_F, N], f32r, tag="xTr")
            nc.vector.tensor_copy(out=xTr, in_=xT_sb)
            ones128 = cpool.tile([1, 128], bf16, tag="ones128")
            nc.vector.memset(ones128, 1.0)
            ones32 = ones128[0:1, 0:HD]
            # a ones row AT partition 32 (matmul lhsT/rhs base partitions
            # must match; e8s lives at partition 32 of ex36)
            ones_p32 = cpool.tile([33, 128], bf16, tag="onesp32")
            nc.vector.memset(ones_p32[32:33, :], 1.0)
            Wr = cpool.tile([IN_F, OUT_F], f32r, tag="Wr")
            nc.scalar.activation(out=Wr, in_=W_sb, func=AF.Copy)
            War = cpool.tile([IN_F, 36], f32r, tag="War")
            nc.scalar.activation(out=War, in_=Wa_sb, func=AF.Copy)

            # identity for the small PE transposes
            ident = cpool.tile([128, 128], bf16, tag="ident")
            make_identity(nc, ident)

            # ---- st rows first (they gate the exp chain): [36, N] ----
            st_ps = pspre.tile([36, N], f32, tag="st")
            for c in range(2):
                nc.tensor.matmul(st_ps[:, 512 * c:512 * (c + 1)], War,
                                 xTr[:, 512 * c:512 * (c + 1)],
                                 start=True, stop=True)
            # ---- hT: [OUT_F, N] = W^T @ x (W stationary, 1 weight load) ----
            hT_ps = pspre.tile([OUT_F, N], f32, tag="hT")
            for c in range(2):
                nc.tensor.matmul(hT_ps[:, 512 * c:512 * (c + 1)], Wr,
                                 xTr[:, 512 * c:512 * (c + 1)],
                                 start=True, stop=True)


            # ONE exp op covers everything (ACT cost is free-dim
            # driven): rows 0-3 exp(t-K), 4-7 exp(-0.8t), 32-35 exp(0.8s)
            ex36 = cpool.tile([36, N], bf16, tag="ex36")
            nc.scalar.activation(out=ex36, in_=st_ps, func=AF.Exp,
                                 scale=scvec[:, 0:1], bias=scvec[:, 1:2])
            et8 = ex36[0:8, :]
            e8s = ex36[32:36, :]
            nc.sync.dma_start(out=e8s_dram[:], in_=e8s)
            # ---- t columns: 8 PE transposes of [8, 128] -> [128, 8] ----
            # etc_ps[:, 8*jt + k]: k=0-3 exp(t-K) heads, k=4-7 exp(-0.8 t)
            etc_ps = pspre.tile([128, NT * 8], bf16, tag="etc")
            for jt in range(NT):
                nc.tensor.transpose(etc_ps[:, 8 * jt:8 * (jt + 1)],
                                    et8[:, 128 * jt:128 * (jt + 1)],
                                    ident[0:8, 0:8])
            etc = cpool.tile([128, NT * 8], f32, tag="etcf")
            nc.vector.tensor_copy(out=etc, in_=etc_ps)
            # hT -> SBUF bf16
            hTs = cpool.tile([OUT_F, N], bf16, tag="hTs")
            nc.scalar.activation(out=hTs, in_=hT_ps, func=AF.Copy)
            wtT = pspre.tile([128, N], bf16, tag="wtT")

            # head 0 broadcast via K=1 PE matmul (no DRAM latency); heads
            # 1-3 via DMA broadcast, hidden under head 0's bulk
            bc0_ps = pspre.tile([128, N], f32, tag="bc0")
            # keep the PE clock ramping while it waits for the exps (the
            # real bc0 matmuls overwrite this with start=True)
            for c in range(2):
                nc.tensor.matmul(bc0_ps[:, 0:512], xTr[:, 0:128],
                                 xTr[:, 0:512], start=True, stop=True)
            for c in range(2):
                nc.tensor.matmul(bc0_ps[:, 512 * c:512 * (c + 1)],
                                 ones_p32[32:33, :],
                                 e8s[0:1, 512 * c:512 * (c + 1)],
                                 start=True, stop=True)
            eb0 = cpool.tile([128, N], bf16, tag="e8sb0")
            nc.scalar.activation(out=eb0, in_=bc0_ps, func=AF.Copy)
            e8s_b = [eb0]
            for h in range(1, H):
                eb = cpool.tile([128, N], bf16, tag=f"e8sb{h}")
                eng = nc.sync if h % 2 == 0 else nc.gpsimd
                eng.dma_start(
                    out=eb, in_=e8s_dram[h:h + 1, :].to_broadcast([128, N]))
                e8s_b.append(eb)

            for c in range(2):
                nc.tensor.matmul(st_ps[0:HD, 512:1024], ones32,
                                 et8[0:1, 512:1024], start=True, stop=True)

            # negated exp(-0.8 t) columns for the ACT tile path's first bias
            etc_v = etc[:].rearrange("p (jt k) -> p jt k", k=8)
            if ACT_TILES:
                netc = cpool.tile([128, H * NT], f32, tag="netc")
                netc_v = netc[:].rearrange("p (jt k) -> p jt k", k=H)
                nc.vector.tensor_scalar_mul(out=netc_v, in0=etc_v[:, :, 4:8],
                                            scalar1=-1.0)

            for jt in range(NT):
                nc.tensor.transpose(wtT[:, 128 * jt:128 * (jt + 1)],
                                    hTs[:, 128 * jt:128 * (jt + 1)],
                                    ident)

            # wt[:, 132*jt + 33*h + c] = h_node[c]*exp(t_j-K) for c<32,
            # col 32 = exp(t_j-K)  (the Z column). Two halves so the first
            # matmuls don't wait on the last xbar transposes.
            wt_all = cpool.tile([128, NT * 33 * H], bf16, tag="wt")
            wt_v = wt_all[:].rearrange("p (jt h c) -> p jt h c", h=H, c=33)
            wtT_v = wtT[:].rearrange("p (jt h c) -> p jt h c", h=H, c=32)
            for half in range(2):
                jts = slice(half * NT // 2, (half + 1) * NT // 2)
                nc.vector.tensor_tensor(
                    out=wt_v[:, jts, :, 0:32], in0=wtT_v[:, jts],
                    in1=etc_v[:, jts, 0:4, None].to_broadcast(
                        [128, NT // 2, H, 32]),
                    op=ALU.mult)
                nc.vector.tensor_copy(out=wt_v[:, jts, :, 32:33],
                                      in_=etc_v[:, jts, 0:4, None])
            wts = [wt_all[:, 132 * jt:132 * (jt + 1)] for jt in range(NT)]

            # bridge the remaining PE idle window before the bulk loop
            # (the clock gate de-ramps on any idle gap; a cold first head
            # costs ~4us) — harmless K=1 rewrites of the retired st_ps
            for c in range(5):
                nc.tensor.matmul(st_ps[0:HD, 0:512], ones32,
                                 et8[0:1, 0:512], start=True, stop=True)

        # ---- main loop + pipelined per-head tails ----
        with (
            tc.tile_pool(name="ps_main", bufs=3, space="PSUM") as psmain,
            tc.tile_pool(name="ps_rzb", bufs=1, space="PSUM") as psrzb,
        ):
            ohs = [None] * H
            deferred = []

            def emit_bulk(h):
                oh = psmain.tile([33, N], f32, tag="oh")
                ohs[h] = oh
                for jt in range(NT):
                    a_t = apool.tile([128, N], bf16, tag="at")
                    if (h, jt) in ACT_TILES:
                        # A'' = relu(e8s - em08t) + em08t via two Relus
                        # (exact: both summands nonnegative)
                        r1 = apool.tile([128, N], bf16, tag="r1")
                        nc.scalar.activation(
                            out=r1, in_=e8s_b[h], func=AF.Relu,
                            bias=netc_v[:, jt, h:h + 1])
                        nc.scalar.activation(
                            out=a_t, in_=r1, func=AF.Relu,
                            bias=etc_v[:, jt, 4 + h:5 + h])
                    else:
                        nc.vector.tensor_scalar_max(
                            out=a_t, in0=e8s_b[h],
                            scalar1=etc_v[:, jt, 4 + h:5 + h])
                    for c in range(2):
                        nc.tensor.matmul(
                            oh[:, 512 * c:512 * (c + 1)],
                            wts[jt][:, 33 * h:33 * (h + 1)],
                            a_t[:, 512 * c:512 * (c + 1)],
                            start=(jt == 0), stop=(jt == NT - 1))

            def emit_tail(h):
                oh = ohs[h]
                # 1/Z = exp(-ln Z) on ACT, straight from the PSUM Z row;
                # rz is partition-broadcast by a K=1 PE matmul (no DMA
                # round trip, so PSUM accumulator slots free up fast)
                o_sb = tpool.tile([HD, N], f32, tag="osb")
                if h == 3:
                    for dh, docp, drzb, dosb in deferred:
                        nc.vector.tensor_tensor(out=dosb, in0=docp,
                                                in1=drzb, op=ALU.mult)
                        nc.sync.dma_start(
                            out=outT[HD * dh:HD * (dh + 1), :], in_=dosb)
                    # everything in 512-halves: each half's ln/exp/rzb/mult
                    # /store pipelines against the other's
                    ocp = tpool.tile([HD, N], f32, tag="ocp")
                    nc.vector.tensor_copy(out=ocp, in_=oh[0:HD, :])
                    warm_ps = psrzb.tile([HD, N], f32, tag="rzbps")
                    for c in range(2):
                        nc.tensor.matmul(warm_ps[:, 0:512], ones32,
                                         et8[0:1, 0:512],
                                         start=True, stop=True)
                    rzb_ps = psrzb.tile([HD, N], f32, tag="rzbps")
                    lnz = tpool.tile([1, N], f32, tag="lnz")
                    rz16 = tpool.tile([1, N], bf16, tag="rz16")
                    for c in range(2):
                        sl = slice(512 * c, 512 * (c + 1))
                        nc.scalar.activation(out=lnz[:, sl],
                                             in_=oh[32:33, sl], func=AF.Ln)
                        nc.scalar.activation(out=rz16[:, sl],
                                             in_=lnz[:, sl], func=AF.Exp,
                                             scale=-1.0)
                        nc.tensor.matmul(rzb_ps[:, sl], ones32,
                                         rz16[:, sl], start=True, stop=True)
                        nc.vector.tensor_tensor(
                            out=o_sb[:, sl], in0=ocp[:, sl],
                            in1=rzb_ps[:, sl], op=ALU.mult)
                        nc.sync.dma_start(
                            out=outT[HD * h:HD * (h + 1), sl],
                            in_=o_sb[:, sl])
                    return
                lnz = tpool.tile([1, N], f32, tag="lnz")
                nc.scalar.activation(out=lnz, in_=oh[32:33, :], func=AF.Ln)
                rz16 = tpool.tile([1, N], bf16, tag="rz16")
                nc.scalar.activation(out=rz16, in_=lnz, func=AF.Exp,
                                     scale=-1.0)
                if h < 2:
                    # free the PSUM slot now (ACT copy), but DEFER the
                    # multiply off the DVE bulk path: 1/Z goes to SBUF via
                    # a DRAM broadcast (latency is irrelevant by then)
                    ocp = tpool.tile([HD, N], f32, tag=f"docp{h}")
                    nc.scalar.activation(out=ocp, in_=oh[0:HD, :],
                                         func=AF.Copy)
                    nc.sync.dma_start(out=rz16_dram[h:h + 1, :], in_=rz16)
                    rzb16 = tpool.tile([HD, N], bf16, tag="rzb16")
                    nc.sync.dma_start(
                        out=rzb16,
                        in_=rz16_dram[h:h + 1, :].to_broadcast([HD, N]))
                    deferred.append((h, ocp, rzb16, o_sb))
                    return
                ocp = tpool.tile([HD, N], f32, tag="ocp")
                nc.vector.tensor_copy(out=ocp, in_=oh[0:HD, :])
                rzb_ps = psrzb.tile([HD, N], f32, tag="rzbps")
                for c in range(2):
                    nc.tensor.matmul(rzb_ps[:, 512 * c:512 * (c + 1)],
                                     ones32,
                                     rz16[:, 512 * c:512 * (c + 1)],
                                     start=True, stop=True)
                nc.vector.tensor_tensor(out=o_sb, in0=ocp, in1=rzb_ps,
                                        op=ALU.mult)
                nc.sync.dma_start(out=outT[HD * h:HD * (h + 1), :], in_=o_sb)

            for h in range(H):
                emit_bulk(h)
                if h >= 1:
                    emit_tail(h - 1)
            emit_tail(H - 1)

    nc.compile()
    return nc


def _get_nc():
    if "nc" not in _CACHE:
        _CACHE["nc"] = _build_nc()
    return _CACHE["nc"]


def _host_inputs(x, W, a_src, a_dst):
    a_ext = np.zeros((OUT_F, 36), np.float32)
    for h in range(H):
        a_ext[h * HD:(h + 1) * HD, h] = a_dst[h]
        a_ext[h * HD:(h + 1) * HD, H + h] = a_dst[h]
        a_ext[h * HD:(h + 1) * HD, 32 + h] = a_src[h]
    Wa = W @ a_ext
    # rows 0-3: exp(1.0*t - K) (wt row factor; -K keeps lnZ inside the
    # ACT Ln LUT's accurate range); rows 4-7: exp(-0.8*t) (bulk max
    # scalar); rows 32-35: exp(0.8*s) (the broadcast tiles)
    scv = np.zeros((36, 2), np.float32)
    scv[0:4, 0] = 1.0
    scv[0:4, 1] = -K_SHIFT
    scv[4:8, 0] = -0.8
    scv[32:36, 0] = 0.8
    return [
        {"xT": np.ascontiguousarray(x[c].T), "W": W, "Wa": Wa, "scv": scv}
        for c in range(N_CORES)
    ]


def kernel(x, W, a_src, a_dst):
    from concourse.bass_utils import run_bass_kernel_spmd

    x = np.asarray(x, dtype=np.float32)
    W = np.asarray(W, dtype=np.float32)
    a_src = np.asarray(a_src, dtype=np.float32)
    a_dst = np.asarray(a_dst, dtype=np.float32)

    nc = _get_nc()
    in_maps = _host_inputs(x, W, a_src, a_dst)
    res = run_bass_kernel_spmd(nc, in_maps, core_ids=list(range(N_CORES)))
    out = np.stack([res.results[c]["outT"].T for c in range(N_CORES)], axis=0)
    return np.ascontiguousarray(out, dtype=np.float32)


# revision 30
# speedup vs baseline: 1.0350x; 1.0350x over previous
"""GAT layer kernel for Trainium2, 8-core data-parallel over batch.

Math (per batch b, head h):
    h = x @ W                              [N, H*HD]
    s_n = <h[n, hHD:(h+1)HD], a_src[h]>,  t_n likewise with a_dst
    A[j, i] = exp(leakyrelu(s_i + t_j, 0.2)),  out[i] = (sum_j A[j,i] h_j) / Z_i

Key identities (softmax over j is invariant to per-i factors; per-j factors
can be folded into the aggregation weights):
    A''[j, i] = max(exp(0.8 s_i), exp(-0.8 t_j))        -- ONE max op per tile
    wt'[j, c] = [h_j | 1][c] * exp(t_j - K)             -- folded j factor
    sum_j wt'[j,c] A''[j,i] = e^{-K-0.2 s_i} sum_j A[j,i] [h_j | 1][c]
so the [33, N] PSUM accumulators hold num and Z up to a per-column factor
that cancels in num/Z. K re-centers ln Z into the ACT Ln LUT's accurate
range (it breaks down above ~40).

Engine plan per core (= one batch element):
  - PE: st rows + hT with stationary weights (f32r), 8 tiny transposes for
    the exp(t) columns, 64 bulk matmuls, head-3's K=1 1/Z broadcast.
  - DVE: 28 of 32 attention tiles (tensor_scalar_max, bf16 4x mode),
    wt scaling, per-head normalize multiplies.
  - ACT: all exps (one scale/bias-AP'd op covers exp(t-K) and exp(-0.8t)),
    hT PSUM->SBUF copy, 4 attention tiles via double-Relu, per-head
    1/Z = Exp(-Ln(Z)) read straight from PSUM.
  - GpSimd: DMA queue driving only (its tensor ops are microcoded-slow AND
    starve DVE through the shared SBUF port).
  - wt transposes ride the DMA xbar transpose engine (SBUF->SBUF).
  - A single ACT table set (natural_log_exp_and_others) serves Copy/Exp/Ln/
    Relu so the tail doesn't thrash table loads (1.28us each).
  - per-head normalize tails are pipelined one head behind the bulk loop;
    head 3 avoids DMA latency via a K=1 PE matmul broadcast of 1/Z.
"""

import numpy as np

B, N, IN_F, OUT_F, H = 8, 1024, 128, 128, 4
HD = OUT_F // H  # 32
NEG = 0.2
K_SHIFT = 14.0  # ln-space downshift of the accumulators (cancels in num/Z)
N_CORES = 8
NT = N // 128  # 8 node tiles

ACT_TILES = {(0, 6), (1, 4)}  # built on ACT (double-Relu) in its idle window

_CACHE = {}


def _patch_act_tables():
    """Restrict Copy/Exp/Ln/Relu/ParametricRelu to the one table set that
    holds them all, so the scheduler emits a single ACT_TABLE_LOAD instead
    of thrashing between per-function tables (1.28us per reload)."""
    import concourse.bacc as bacc_mod
    if getattr(bacc_mod, "_gat_tables_patched", False):
        return
    orig = bacc_mod.get_activation_tables

    def patched(arch):
        tables = orig(arch)
        combined = "natural_log_exp_and_others"
        if combined in tables:
            keep = tables[combined]
            for name, funcs in tables.items():
                if name != combined:
                    funcs -= keep
        return tables

    bacc_mod.get_activation_tables = patched
    bacc_mod._gat_tables_patched = True


def _build_nc():
    _patch_act_tables()
    import concourse.bacc as bacc
    import concourse.tile as tile
    from concourse import mybir
    from concourse.masks import make_identity

    f32 = mybir.dt.float32
    f32r = mybir.dt.float32r
    bf16 = mybir.dt.bfloat16
    AF = mybir.ActivationFunctionType
    ALU = mybir.AluOpType

    nc = bacc.Bacc("TRN2", target_bir_lowering=False, debug=False,
                   num_devices=N_CORES)

    xT = nc.declare_dram_parameter("xT", [IN_F, N], f32, isOutput=False)
    Wd = nc.declare_dram_parameter("W", [IN_F, OUT_F], f32, isOutput=False)
    # Wa_ext = W @ [a_dst | a_dst | 0... | a_src]: st rows 0-7 = t twice,
    # rows 32-35 = s (32-aligned partition offsets for the ACT reads)
    Wa = nc.declare_dram_parameter("Wa", [IN_F, 36], f32, isOutput=False)
    scv_d = nc.declare_dram_parameter("scv", [36, 2], f32, isOutput=False)
    outT = nc.declare_dram_parameter("outT", [OUT_F, N], f32, isOutput=True)

    e8s_dram = nc.dram_tensor("e8s_scratch", [H, N], bf16)
    rz16_dram = nc.dram_tensor("rz16_scratch", [2, N], bf16)

    with tile.TileContext(nc) as tc:
      with (
        tc.tile_pool(name="const", bufs=1) as cpool,
        tc.tile_pool(name="atile", bufs=10) as apool,
        tc.tile_pool(name="tail", bufs=2) as tpool,
      ):
        with tc.tile_pool(name="ps_pre", bufs=1, space="PSUM") as pspre:
            # ---- load inputs ----
            xT_sb = cpool.tile([IN_F, N], f32, tag="xT")
            nc.sync.dma_start(out=xT_sb[:, 0:512], in_=xT[:, 0:512])
            nc.gpsimd.dma_start(out=xT_sb[:, 512:N], in_=xT[:, 512:N])
            W_sb = cpool.tile([IN_F, OUT_F], f32, tag="W")
            nc.sync.dma_start(out=W_sb, in_=Wd[:])
            Wa_sb = cpool.tile([IN_F, 36], f32, tag="Wa")
            nc.sync.dma_start(out=Wa_sb, in_=Wa[:])
            scvec = cpool.tile([36, 2], f32, tag="scvec")
            nc.sync.dma_start(out=scvec, in_=scv_d[:])

            # f32->f32r rounding casts (fp32 matmul is 1/4 rate; f32r
            # streams 1 col/cyc and the verifier wants explicit rounding)
            xTr = cpool.tile([IN_F, N], f32r, tag="xTr")
            nc.vector.tensor_copy(out=xTr, in_=xT_sb)
            ones128 = cpool.tile([1, 128], bf16, tag="ones128")
            nc.vector.memset(ones128, 1.0)
            ones32 = ones128[0:1, 0:HD]
            # a ones row AT partition 32 (matmul lhsT/rhs base partitions
            # must match; e8s lives at partition 32 of ex36)
            ones_p32 = cpool.tile([33, 128], bf16, tag="onesp32")
            nc.vector.memset(ones_p32[32:33, :], 1.0)
            Wr = cpool.tile([IN_F, OUT_F], f32r, tag="Wr")
            nc.scalar.activation(out=Wr, in_=W_sb, func=AF.Copy)
            War = cpool.tile([IN_F, 36], f32r, tag="War")
            nc.scalar.activation(out=War, in_=Wa_sb, func=AF.Copy)

            # identity for the small PE transposes
            ident = cpool.tile([128, 128], bf16, tag="ident")
            make_identity(nc, ident)

            # ---- st rows first (they gate the exp chain): [36, N] ----
            st_ps = pspre.tile([36, N], f32, tag="st")
            for c in range(2):
                nc.tensor.matmul(st_ps[:, 512 * c:512 * (c + 1)], War,
                                 xTr[:, 512 * c:512 * (c + 1)],
                                 start=True, stop=True)
            # ---- hT: [OUT_F, N] = W^T @ x (W stationary, 1 weight load) ----
            hT_ps = pspre.tile([OUT_F, N], f32, tag="hT")
            for c in range(2):
                nc.tensor.matmul(hT_ps[:, 512 * c:512 * (c + 1)], Wr,
                                 xTr[:, 512 * c:512 * (c + 1)],
                                 start=True, stop=True)


            # ONE exp op covers everything (ACT cost is free-dim
            # driven): rows 0-3 exp(t-K), 4-7 exp(-0.8t), 32-35 exp(0.8s)
            ex36 = cpool.tile([36, N], bf16, tag="ex36")
            nc.scalar.activation(out=ex36, in_=st_ps, func=AF.Exp,
                                 scale=scvec[:, 0:1], bias=scvec[:, 1:2])
            et8 = ex36[0:8, :]
            e8s = ex36[32:36, :]
            nc.sync.dma_start(out=e8s_dram[:], in_=e8s)
            # ---- t columns: 8 PE transposes of [8, 128] -> [128, 8] ----
            # etc_ps[:, 8*jt + k]: k=0-3 exp(t-K) heads, k=4-7 exp(-0.8 t)
            etc_ps = pspre.tile([128, NT * 8], bf16, tag="etc")
            for jt in range(NT):
                nc.tensor.transpose(etc_ps[:, 8 * jt:8 * (jt + 1)],
                                    et8[:, 128 * jt:128 * (jt + 1)],
                                    ident[0:8, 0:8])
            etc = cpool.tile([128, NT * 8], f32, tag="etcf")
            nc.vector.tensor_copy(out=etc, in_=etc_ps)
            # hT -> SBUF bf16
            hTs = cpool.tile([OUT_F, N], bf16, tag="hTs")
            nc.scalar.activation(out=hTs, in_=hT_ps, func=AF.Copy)
            wtT = pspre.tile([128, N], bf16, tag="wtT")

            # head 0 broadcast via K=1 PE matmul (no DRAM latency); heads
            # 1-3 via DMA broadcast, hidden under head 0's bulk
            bc0_ps = pspre.tile([128, N], f32, tag="bc0")
            # keep the PE clock ramping while it waits for the exps (the
            # real bc0 matmuls overwrite this with start=True)
            for c in range(2):
                nc.tensor.matmul(bc0_ps[:, 0:512], xTr[:, 0:128],
                                 xTr[:, 0:512], start=True, stop=True)
            for c in range(2):
                nc.tensor.matmul(bc0_ps[:, 512 * c:512 * (c + 1)],
                                 ones_p32[32:33, :],
                                 e8s[0:1, 512 * c:512 * (c + 1)],
                                 start=True, stop=True)
            eb0 = cpool.tile([128, N], bf16, tag="e8sb0")
            nc.scalar.activation(out=eb0, in_=bc0_ps, func=AF.Copy)
            e8s_b = [eb0]
            for h in range(1, H):
                eb = cpool.tile([128, N], bf16, tag=f"e8sb{h}")
                eng = nc.sync if h % 2 == 0 else nc.gpsimd
                eng.dma_start(
                    out=eb, in_=e8s_dram[h:h + 1, :].to_broadcast([128, N]))
                e8s_b.append(eb)

            for c in range(2):
                nc.tensor.matmul(st_ps[0:HD, 512:1024], ones32,
                                 et8[0:1, 512:1024], start=True, stop=True)

            # negated exp(-0.8 t) columns for the ACT tile path's first bias
            etc_v = etc[:].rearrange("p (jt k) -> p jt k", k=8)
            if ACT_TILES:
                netc = cpool.tile([128, H * NT], f32, tag="netc")
                netc_v = netc[:].rearrange("p (jt k) -> p jt k", k=H)
                nc.vector.tensor_scalar_mul(out=netc_v, in0=etc_v[:, :, 4:8],
                                            scalar1=-1.0)

            for jt in range(NT):
                nc.tensor.transpose(wtT[:, 128 * jt:128 * (jt + 1)],
                                    hTs[:, 128 * jt:128 * (jt + 1)],
                                    ident)

            # wt[:, 132*jt + 33*h + c] = h_node[c]*exp(t_j-K) for c<32,
            # col 32 = exp(t_j-K)  (the Z column). Two halves so the first
            # matmuls don't wait on the last xbar transposes.
            wt_all = cpool.tile([128, NT * 33 * H], bf16, tag="wt")
            wt_v = wt_all[:].rearrange("p (jt h c) -> p jt h c", h=H, c=33)
            wtT_v = wtT[:].rearrange("p (jt h c) -> p jt h c", h=H, c=32)
            for half in range(2):
                jts = slice(half * NT // 2, (half + 1) * NT // 2)
                nc.vector.tensor_tensor(
                    out=wt_v[:, jts, :, 0:32], in0=wtT_v[:, jts],
                    in1=etc_v[:, jts, 0:4, None].to_broadcast(
                        [128, NT // 2, H, 32]),
                    op=ALU.mult)
                nc.vector.tensor_copy(out=wt_v[:, jts, :, 32:33],
                                      in_=etc_v[:, jts, 0:4, None])
            wts = [wt_all[:, 132 * jt:132 * (jt + 1)] for jt in range(NT)]

            # bridge the remaining PE idle window before the bulk loop
            # (the clock gate de-ramps on any idle gap; a cold first head
            # costs ~4us) — harmless K=1 rewrites of the retired st_ps
            for c in range(5):
                nc.tensor.matmul(st_ps[0:HD, 0:512], ones32,
                                 et8[0:1, 0:512], start=True, stop=True)

        # ---- main loop + pipelined per-head tails ----
        with (
            tc.tile_pool(name="ps_main", bufs=3, space="PSUM") as psmain,
            tc.tile_pool(name="ps_rzb", bufs=1, space="PSUM") as psrzb,
        ):
            ohs = [None] * H
            deferred = []

            def emit_bulk(h):
                oh = psmain.tile([33, N], f32, tag="oh")
                ohs[h] = oh
                for jt in range(NT):
                    a_t = apool.tile([128, N], bf16, tag="at")
                    if (h, jt) in ACT_TILES:
                        # A'' = relu(e8s - em08t) + em08t via two Relus
                        # (exact: both summands nonnegative)
                        r1 = apool.tile([128, N], bf16, tag="r1")
                        nc.scalar.activation(
                            out=r1, in_=e8s_b[h], func=AF.Relu,
                            bias=netc_v[:, jt, h:h + 1])
                        nc.scalar.activation(
                            out=a_t, in_=r1, func=AF.Relu,
                            bias=etc_v[:, jt, 4 + h:5 + h])
                    else:
                        nc.vector.tensor_scalar_max(
                            out=a_t, in0=e8s_b[h],
                            scalar1=etc_v[:, jt, 4 + h:5 + h])
                    for c in range(2):
                        nc.tensor.matmul(
                            oh[:, 512 * c:512 * (c + 1)],
                            wts[jt][:, 33 * h:33 * (h + 1)],
                            a_t[:, 512 * c:512 * (c + 1)],
                            start=(jt == 0), stop=(jt == NT - 1))

            def emit_tail(h):
                oh = ohs[h]
                # 1/Z = exp(-ln Z) on ACT, straight from the PSUM Z row;
                # rz is partition-broadcast by a K=1 PE matmul (no DMA
                # round trip, so PSUM accumulator slots free up fast)
                o_sb = tpool.tile([HD, N], f32,
                                  tag=f"dosb{h}" if h < 2 else "osb")
                if h == 3:
                    for dh, docp, drzb, dosb in deferred:
                        nc.vector.tensor_tensor(out=dosb, in0=docp,
                                                in1=drzb, op=ALU.mult)
                        nc.sync.dma_start(
                            out=outT[HD * dh:HD * (dh + 1), :], in_=dosb)
                    # everything in 512-halves: each half's ln/exp/rzb/mult
                    # /store pipelines against the other's
                    ocp = tpool.tile([HD, N], f32, tag="ocp")
                    nc.vector.tensor_copy(out=ocp, in_=oh[0:HD, :])
                    warm_ps = psrzb.tile([HD, N], f32, tag="rzbps")
                    for c in range(2):
                        nc.tensor.matmul(warm_ps[:, 0:512], ones32,
                                         et8[0:1, 0:512],
                                         start=True, stop=True)
                    rzb_ps = psrzb.tile([HD, N], f32, tag="rzbps")
                    lnz = tpool.tile([1, N], f32, tag="lnz")
                    rz16 = tpool.tile([1, N], bf16, tag="rz16")
                    for c in range(2):
                        sl = slice(512 * c, 512 * (c + 1))
                        nc.scalar.activation(out=lnz[:, sl],
                                             in_=oh[32:33, sl], func=AF.Ln)
                        nc.scalar.activation(out=rz16[:, sl],
                                             in_=lnz[:, sl], func=AF.Exp,
                                             scale=-1.0)
                        nc.tensor.matmul(rzb_ps[:, sl], ones32,
                                         rz16[:, sl], start=True, stop=True)
                        nc.vector.tensor_tensor(
                            out=o_sb[:, sl], in0=ocp[:, sl],
                            in1=rzb_ps[:, sl], op=ALU.mult)
                        nc.sync.dma_start(
                            out=outT[HD * h:HD * (h + 1), sl],
                            in_=o_sb[:, sl])
                    return
                lnz = tpool.tile([1, N], f32, tag="lnz")
                nc.scalar.activation(out=lnz, in_=oh[32:33, :], func=AF.Ln)
                rz16 = tpool.tile([1, N], bf16, tag="rz16")
                nc.scalar.activation(out=rz16, in_=lnz, func=AF.Exp,
                                     scale=-1.0)
                if h < 2:
                    # free the PSUM slot now (ACT copy), but DEFER the
                    # multiply off the DVE bulk path: 1/Z goes to SBUF via
                    # a DRAM broadcast (latency is irrelevant by then)
                    ocp = tpool.tile([HD, N], f32, tag=f"docp{h}")
                    nc.scalar.activation(out=ocp, in_=oh[0:HD, :],
                                         func=AF.Copy)
                    nc.sync.dma_start(out=rz16_dram[h:h + 1, :], in_=rz16)
                    rzb16 = tpool.tile([HD, N], bf16, tag="rzb16")
                    nc.sync.dma_start(
                        out=rzb16,
                        in_=rz16_dram[h:h + 1, :].to_broadcast([HD, N]))
                    deferred.append((h, ocp, rzb16, o_sb))
                    return
                ocp = tpool.tile([HD, N], f32, tag="ocp")
                nc.vector.tensor_copy(out=ocp, in_=oh[0:HD, :])
                rzb_ps = psrzb.tile([HD, N], f32, tag="rzbps")
                for c in range(2):
                    nc.tensor.matmul(rzb_ps[:, 512 * c:512 * (c + 1)],
                                     ones32,
                                     rz16[:, 512 * c:512 * (c + 1)],
                                     start=True, stop=True)
                nc.vector.tensor_tensor(out=o_sb, in0=ocp, in1=rzb_ps,
                                        op=ALU.mult)
                nc.sync.dma_start(out=outT[HD * h:HD * (h + 1), :], in_=o_sb)

            for h in range(H):
                emit_bulk(h)
                if h >= 1:
                    emit_tail(h - 1)
            emit_tail(H - 1)

    nc.compile()
    return nc


def _get_nc():
    if "nc" not in _CACHE:
        _CACHE["nc"] = _build_nc()
    return _CACHE["nc"]


def _host_inputs(x, W, a_src, a_dst):
    a_ext = np.zeros((OUT_F, 36), np.float32)
    for h in range(H):
        a_ext[h * HD:(h + 1) * HD, h] = a_dst[h]
        a_ext[h * HD:(h + 1) * HD, H + h] = a_dst[h]
        a_ext[h * HD:(h + 1) * HD, 32 + h] = a_src[h]
    Wa = W @ a_ext
    # rows 0-3: exp(1.0*t - K) (wt row factor; -K keeps lnZ inside the
    # ACT Ln LUT's accurate range); rows 4-7: exp(-0.8*t) (bulk max
    # scalar); rows 32-35: exp(0.8*s) (the broadcast tiles)
    scv = np.zeros((36, 2), np.float32)
    scv[0:4, 0] = 1.0
    scv[0:4, 1] = -K_SHIFT
    scv[4:8, 0] = -0.8
    scv[32:36, 0] = 0.8
    return [
        {"xT": np.ascontiguousarray(x[c].T), "W": W, "Wa": Wa, "scv": scv}
        for c in range(N_CORES)
    ]


def kernel(x, W, a_src, a_dst):
    from concourse.bass_utils import run_bass_kernel_spmd

    x = np.asarray(x, dtype=np.float32)
    W = np.asarray(W, dtype=np.float32)
    a_src = np.asarray(a_src, dtype=np.float32)
    a_dst = np.asarray(a_dst, dtype=np.float32)

    nc = _get_nc()
    in_maps = _host_inputs(x, W, a_src, a_dst)
    res = run_bass_kernel_spmd(nc, in_maps, core_ids=list(range(N_CORES)))
    out = np.stack([res.results[c]["outT"].T for c in range(N_CORES)], axis=0)
    return np.ascontiguousarray(out, dtype=np.float32)
